# revision 9
# baseline (speedup 1.0000x reference)
"""Bass/Trainium2 kernel for nn_CopulaModel: 2-layer GRU copula loss.

Self-contained: kernel(**inputs) takes FULL inputs, shards batch-parallel
across 8 NeuronCores, runs one SPMD Bass program, returns the scalar loss.

Math: sigma = diag(d) + V V^T (V: [T=256, R=64]) so via Woodbury +
matrix-determinant-lemma everything reduces to the 64x64 capacitance
C = I + V^T D^-1 V per batch:
  quad   = r^T D^-1 r - u^T C^-1 u,   u = V^T D^-1 r
  logdet = sum log d + log det C
C^-1 u and log det C come from one augmented Gaussian elimination on
[C | u] whose pivots D_j give logdet and whose transformed u gives
u^T C^-1 u = sum utilde_j^2 / D_j.
"""
import os, sys, types
import numpy as np
import ml_dtypes

# ---- axon runtime setup (the image's antenv lacks the NTFF hook) ----
if "/root/.axon_site" not in sys.path:
    sys.path.insert(0, "/root/.axon_site")
if "antenv.axon_hooks" not in sys.modules:
    try:
        from trn_agent_boot.trn_boot import _ntff_profile_via_ctypes
        _hook = _ntff_profile_via_ctypes("/opt/axon/libaxon_pjrt.so")
    except Exception:
        _hook = None
    _m = types.ModuleType("antenv.axon_hooks")
    _m.get_axon_ntff_profile_hook = lambda: _hook
    _m.set_axon_ntff_profile_hook = lambda h: None
    sys.modules["antenv.axon_hooks"] = _m

from contextlib import ExitStack
import concourse.bacc as bacc
import concourse.tile as tile
from concourse import mybir
import concourse.bass_utils as _bu
_bu.upload_artifacts = lambda d: f"local:{d}"
from concourse.bass_utils import run_bass_kernel_spmd

F32 = mybir.dt.float32
BF16 = mybir.dt.bfloat16
AF = mybir.ActivationFunctionType
ALU = mybir.AluOpType
AX = mybir.AxisListType

B, T, L, H, R = 32, 256, 64, 64, 64
NCORES = 8
NB = B // NCORES            # batches per core = 4
N = NB * T                  # GRU rows per core = 1024
CH = 512                    # free-dim chunk
W65 = R + 1                 # augmented width per batch
AW = NB * W65               # 260

_CACHE = {}
last_exec_time_ns = None


def _build():
    nc = bacc.Bacc("TRN2", target_bir_lowering=False, debug=False,
                   num_devices=NCORES)
    d = {}
    d["x3"] = nc.dram_tensor("x3", [L, 3, N], BF16, kind="ExternalInput")
    d["yp"] = nc.dram_tensor("yp", [1, N], F32, kind="ExternalInput")
    for nm in ("sh1_r", "sh1_z", "sh1_n", "sx2_r", "sx2_z", "sx2_n",
               "sh2_r", "sh2_z", "sh2_n"):
        d[nm] = nc.dram_tensor(nm, [128, 128], BF16, kind="ExternalInput")
    for nm in ("sx1_r", "sx1_z", "sx1_n"):
        d[nm] = nc.dram_tensor(nm, [3, 128], BF16, kind="ExternalInput")
    for nm in ("b_r2", "b_z2", "b_hn1", "b_hn2", "b_in2"):
        d[nm] = nc.dram_tensor(nm, [128, 1], F32, kind="ExternalInput")
    d["identb"] = nc.dram_tensor("identb", [128, 128], BF16, kind="ExternalInput")
    d["sv"] = nc.dram_tensor("sv", [128, R], BF16, kind="ExternalInput")
    d["smd"] = nc.dram_tensor("smd", [128, 2], BF16, kind="ExternalInput")
    d["vb"] = nc.dram_tensor("vb", [R, 1], F32, kind="ExternalInput")
    d["id65"] = nc.dram_tensor("id65", [W65, W65], F32, kind="ExternalInput")
    d["iaug"] = nc.dram_tensor("iaug", [R, W65], F32, kind="ExternalInput")
    d["db"] = nc.dram_tensor("db", [1, 1], F32, kind="ExternalInput")
    out_d = nc.dram_tensor("out", [1, 1], F32, kind="ExternalOutput")

    with tile.TileContext(nc) as tc, ExitStack() as stack:
        const = stack.enter_context(tc.tile_pool(name="const", bufs=1))
        cst = {}
        for nm, dt_, shp in [
            ("sh1_r", BF16, [128, 128]), ("sh1_z", BF16, [128, 128]),
            ("sh1_n", BF16, [128, 128]), ("sx2_r", BF16, [128, 128]),
            ("sx2_z", BF16, [128, 128]), ("sx2_n", BF16, [128, 128]),
            ("sh2_r", BF16, [128, 128]), ("sh2_z", BF16, [128, 128]),
            ("sh2_n", BF16, [128, 128]), ("sx1_r", BF16, [3, 128]),
            ("sx1_z", BF16, [3, 128]), ("sx1_n", BF16, [3, 128]),
            ("b_r2", F32, [128, 1]), ("b_z2", F32, [128, 1]),
            ("b_hn1", F32, [128, 1]), ("b_hn2", F32, [128, 1]),
            ("b_in2", F32, [128, 1]), ("identb", BF16, [128, 128]),
            ("sv", BF16, [128, R]), ("smd", BF16, [128, 2]),
            ("vb", F32, [R, 1]), ("id65", F32, [W65, W65]),
            ("iaug", F32, [R, W65]), ("yp", F32, [1, N]),
            ("db", F32, [1, 1]),
        ]:
            cst[nm] = const.tile(shp, dt_, name=nm, tag=nm)
            nc.sync.dma_start(cst[nm][:], d[nm][:])

        persist = stack.enter_context(tc.tile_pool(name="persist", bufs=1))
        hpool = stack.enter_context(tc.tile_pool(name="h", bufs=3))
        work = stack.enter_context(tc.tile_pool(name="work", bufs=3))

        # ---------------- Phase A: stacked GRU over 64 lags ----------
        h1 = hpool.tile([128, N], BF16, tag="h1")
        h2 = hpool.tile([128, N], BF16, tag="h2")
        nc.vector.memset(h1[:], 0.0)
        nc.vector.memset(h2[:], 0.0)

        with (
            tc.tile_pool(name="ps_rz", bufs=2, space="PSUM") as ps_rz,
            tc.tile_pool(name="ps_g", bufs=2, space="PSUM") as ps_g,
        ):
            for t in range(L):
                h1n = hpool.tile([128, N], BF16, tag="h1")
                h2n = hpool.tile([128, N], BF16, tag="h2")
                for layer in (1, 2):
                    h_prev = h1 if layer == 1 else h2
                    h_new = h1n if layer == 1 else h2n
                    sh_r, sh_z, sh_n = (cst[f"sh{layer}_r"], cst[f"sh{layer}_z"],
                                        cst[f"sh{layer}_n"])
                    sx_r, sx_z, sx_n = (cst[f"sx{layer}_r"], cst[f"sx{layer}_z"],
                                        cst[f"sx{layer}_n"])
                    blocks = []
                    for c in range(N // CH):
                        sl = slice(c * CH, (c + 1) * CH)
                        if layer == 1:
                            xp = work.tile([3, CH], BF16, tag=f"xp{c}")
                            nc.sync.dma_start(xp[:], d["x3"][t, :, sl])
                            xmov = xp[:]
                        else:
                            xmov = h1n[:, sl]
                        rz = ps_rz.tile([128, 2 * CH], F32, tag="rz")
                        gh = ps_g.tile([128, CH], F32, tag="gh")
                        gx = ps_g.tile([128, CH], F32, tag="gx")
                        nc.tensor.matmul(rz[:, 0:CH], sh_r[:], h_prev[:, sl],
                                         start=True, stop=False)
                        nc.tensor.matmul(rz[:, 0:CH], sx_r[:], xmov,
                                         start=False, stop=True)
                        nc.tensor.matmul(rz[:, CH:2 * CH], sh_z[:], h_prev[:, sl],
                                         start=True, stop=False)
                        nc.tensor.matmul(rz[:, CH:2 * CH], sx_z[:], xmov,
                                         start=False, stop=True)
                        nc.tensor.matmul(gh[:], sh_n[:], h_prev[:, sl],
                                         start=True, stop=True)
                        nc.tensor.matmul(gx[:], sx_n[:], xmov,
                                         start=True, stop=False)
                        blocks.append((sl, rz, gh, gx))
                    for sl, rz, gh, gx in blocks:
                        rzs = work.tile([128, 2 * CH], BF16, tag="rzs")
                        if layer == 1:
                            nc.scalar.activation(rzs[:], rz[:], AF.Sigmoid)
                        else:
                            nc.scalar.activation(rzs[:, 0:CH], rz[:, 0:CH],
                                                 AF.Sigmoid, bias=cst["b_r2"][:])
                            nc.scalar.activation(rzs[:, CH:2 * CH], rz[:, CH:2 * CH],
                                                 AF.Sigmoid, bias=cst["b_z2"][:])
                        bb = work.tile([128, CH], BF16, tag="bb")
                        nc.vector.scalar_tensor_tensor(
                            bb[:], gh[:], cst[f"b_hn{layer}"][:], rzs[:, 0:CH],
                            ALU.add, ALU.mult)
                        nc.tensor.matmul(gx[:], cst["identb"][:], bb[:],
                                         start=False, stop=True)
                        nn = work.tile([128, CH], BF16, tag="nn")
                        if layer == 1:
                            nc.scalar.activation(nn[:], gx[:], AF.Tanh)
                        else:
                            nc.scalar.activation(nn[:], gx[:], AF.Tanh,
                                                 bias=cst["b_in2"][:])
                        tt = work.tile([128, CH], BF16, tag="tt")
                        uu = work.tile([128, CH], BF16, tag="uu")
                        nc.vector.tensor_tensor(tt[:], h_prev[:, sl], nn[:],
                                                ALU.subtract)
                        nc.vector.tensor_tensor(uu[:], rzs[:, CH:2 * CH], tt[:],
                                                ALU.mult)
                        nc.vector.tensor_tensor(h_new[:, sl], nn[:], uu[:],
                                                ALU.add)
                h1, h2 = h1n, h2n

        # ---------------- Phase B: v/mu/d + capacitance assembly -----
        T1 = persist.tile([W65 + 1, N], F32, tag="T1")   # 66 rows: V | resid | pad
        T2 = persist.tile([W65 + 1, N], F32, tag="T2")   # W=(D^-1 V) | s
        d_sb = persist.tile([1, N], F32, tag="d_sb")
        dinv = persist.tile([1, N], F32, tag="dinv")
        A = persist.tile([R, AW], F32, tag="A")
        qd = persist.tile([1, NB], F32, tag="qd")
        qd64 = persist.tile([R + 1, NB], F32, tag="qd64")
        with (
            tc.tile_pool(name="ps_v", bufs=1, space="PSUM") as ps_v,
            tc.tile_pool(name="ps_md", bufs=1, space="PSUM") as ps_md,
        ):
            vtp = ps_v.tile([R, N], F32, tag="vtp")
            mdp_mu = ps_md.tile([1, N], F32, tag="mdp_mu")
            mdp_d = ps_md.tile([1, N], F32, tag="mdp_d")
            for c in range(N // CH):
                sl = slice(c * CH, (c + 1) * CH)
                nc.tensor.matmul(vtp[:, sl], cst["sv"][:], h2[:, sl],
                                 start=True, stop=True)
                nc.tensor.matmul(mdp_mu[:, sl], cst["smd"][:, 0:1], h2[:, sl],
                                 start=True, stop=True)
                nc.tensor.matmul(mdp_d[:, sl], cst["smd"][:, 1:2], h2[:, sl],
                                 start=True, stop=True)
                nc.scalar.activation(T1[0:R, sl], vtp[:, sl], AF.Identity,
                                     bias=cst["vb"][:])
            # resid = yp - mu  (yp already has mean_b subtracted)
            rs = persist.tile([1, N], F32, name="rs", tag="rs")
            nc.vector.scalar_tensor_tensor(rs[:], mdp_mu[0:1, :], -1.0,
                                           cst["yp"][:], ALU.mult, ALU.add)
            # d = softplus(dpre + d_b); dinv = 1/d
            edp = persist.tile([1, N], F32, name="edp", tag="edp")
            nc.scalar.activation(edp[:], mdp_d[0:1, :], AF.Exp,
                                 bias=cst["db"][0:1, :])
            nc.scalar.activation(d_sb[:], edp[:], AF.Ln, bias=1.0)
            nc.vector.reciprocal(dinv[:], d_sb[:])
            dB = persist.tile([R, N], F32, tag="dB")
            nc.gpsimd.partition_broadcast(dB[:], dinv[0:1, :])
            nc.vector.tensor_tensor(T2[0:R, :], T1[0:R, :], dB[:], ALU.mult)
            ss = persist.tile([1, N], F32, name="ss", tag="ss")
            nc.vector.tensor_tensor(ss[:], rs[:], dinv[0:1, :], ALU.mult)
            nc.sync.dma_start(T1[R:R + 1, :], rs[:])
            nc.sync.dma_start(T2[R:R + 1, :], ss[:])
        with (
            tc.tile_pool(name="ps_t", bufs=2, space="PSUM") as ps_t,
            tc.tile_pool(name="ps_cu", bufs=2, space="PSUM") as ps_cu,
        ):
            # transpose both [65, 128]-chunks -> [128, 65]
            v1c, v2c = [], []
            for c in range(N // 128):
                slc = slice(c * 128, (c + 1) * 128)
                tp1 = ps_t.tile([128, W65], F32, tag="tp1")
                nc.tensor.transpose(tp1[:], T1[0:W65, slc], cst["id65"][:])
                s1 = work.tile([128, W65], F32, tag=f"v1c{c % 2}")
                nc.scalar.copy(s1[:], tp1[:])
                tp2 = ps_t.tile([128, W65], F32, tag="tp2")
                nc.tensor.transpose(tp2[:], T2[0:W65, slc], cst["id65"][:])
                s2 = work.tile([128, W65], F32, tag=f"v2c{c % 2}")
                nc.scalar.copy(s2[:], tp2[:])
                v1c.append(s1)
                v2c.append(s2)
            for b in range(NB):
                cu = ps_cu.tile([W65, W65], F32, tag="cu")
                nc.tensor.matmul(cu[:], v2c[2 * b][:], v1c[2 * b][:],
                                 start=True, stop=False)
                nc.tensor.matmul(cu[:], v2c[2 * b + 1][:], v1c[2 * b + 1][:],
                                 start=False, stop=True)
                nc.vector.tensor_tensor(A[:, b * W65:(b + 1) * W65],
                                        cu[0:R, :], cst["iaug"][:], ALU.add)
                nc.vector.tensor_copy(qd64[R:R + 1, b:b + 1],
                                      cu[R:R + 1, R:R + 1])

        nc.sync.dma_start(qd[:], qd64[R:R + 1, :])

        # ---------------- Phase C: augmented elimination -------------
        Rinv = persist.tile([R, NB * R], F32, tag="Rinv")   # [64, 256], col 4j+b
        Util = persist.tile([R, NB * R], F32, tag="Util")
        elim = stack.enter_context(tc.tile_pool(name="elim", bufs=3))
        Ar = A[:].rearrange("p (b c) -> p b c", b=NB)        # [64, 4, 65]
        for j in range(R):
            r0 = elim.tile([1, AW], F32, tag="r0")
            nc.sync.dma_start(r0[:], A[j:j + 1, :])
            bc = elim.tile([R, AW], F32, tag="bc")
            nc.gpsimd.partition_broadcast(bc[:], r0[0:1, :])
            bcr = bc[:].rearrange("p (b c) -> p b c", b=NB)
            rv = Rinv[:, NB * j:NB * (j + 1)].rearrange("p (b o) -> p b o", o=1)
            # rinv_neg = -1/pivot  (pivot = bc[:, b, j], same on all partitions)
            npv = elim.tile([R, NB], F32, name="npv", tag="npv")
            npr = npv[:].rearrange("p (b o) -> p b o", o=1)
            nc.vector.tensor_scalar(npr, bcr[:, :, j:j + 1], -1.0, None,
                                    ALU.mult)
            nc.vector.reciprocal(rv, npr)
            ut = Util[:, NB * j:NB * (j + 1)].rearrange("p (b o) -> p b o", o=1)
            nc.vector.tensor_copy(ut, bcr[:, :, R:R + 1])
            w = elim.tile([R, NB], F32, tag="w")
            wr = w[:].rearrange("p (b o) -> p b o", o=1)
            nc.vector.tensor_tensor(wr, Ar[:, :, j:j + 1], rv, ALU.mult)
            for b in range(NB):
                sl = slice(b * W65, (b + 1) * W65)
                nc.vector.scalar_tensor_tensor(
                    A[:, sl], bc[:, sl], w[:, b:b + 1], A[:, sl],
                    ALU.mult, ALU.add)

        # ---------------- Phase D: reduce to the scalar loss ---------
        fin = stack.enter_context(tc.tile_pool(name="fin", bufs=1))
        t1 = fin.tile([1, NB * R], F32, tag="t1")
        nc.vector.tensor_tensor(t1[:], Util[0:1, :], Util[0:1, :], ALU.mult)
        nc.vector.tensor_tensor(t1[:], t1[:], Rinv[0:1, :], ALU.mult)
        t2r = fin.tile([1, NB], F32, tag="t2r")
        nc.vector.tensor_reduce(t2r[:], t1[0:1, :].rearrange(
            "p (j b) -> p b j", b=NB), AX.X, ALU.add)
        t3 = fin.tile([1, NB * R], F32, tag="t3")
        nc.scalar.activation(t3[:], Rinv[0:1, :], AF.Ln, scale=-1.0)
        t3r = fin.tile([1, NB], F32, tag="t3r")
        nc.vector.tensor_reduce(t3r[:], t3[0:1, :].rearrange(
            "p (j b) -> p b j", b=NB), AX.X, ALU.add)
        logd = fin.tile([1, N], F32, tag="logd")
        nc.scalar.activation(logd[:], d_sb[:], AF.Ln)
        ldr = fin.tile([1, NB], F32, tag="ldr")
        nc.vector.tensor_reduce(ldr[:], logd[0:1, :].rearrange(
            "p (b t) -> p b t", b=NB), AX.X, ALU.add)
        acc = fin.tile([1, NB], F32, tag="acc")
        nc.vector.tensor_tensor(acc[:], qd[:], t2r[:], ALU.add)
        nc.vector.tensor_tensor(acc[:], acc[:], ldr[:], ALU.add)
        nc.vector.tensor_tensor(acc[:], acc[:], t3r[:], ALU.subtract)
        tot = fin.tile([1, 1], F32, tag="tot")
        nc.vector.tensor_reduce(tot[:], acc[0:1, :], AX.X, ALU.add)
        nc.sync.dma_start(out_d[:], tot[:])

    nc.compile()
    return nc


def _gate(a, g):
    return a[g * H:(g + 1) * H]


def _consts(params):
    p = {k: (np.asarray(v, np.float32) if not isinstance(v, list) else
             [{kk: np.asarray(vv, np.float32) for kk, vv in lay.items()}
              for lay in v]) for k, v in params.items()}
    gl, gr = p["gru_left"], p["gru_right"]
    c = {}
    for layer in (1, 2):
        pl, pr = gl[layer - 1], gr[layer - 1]
        for gi, gn in enumerate("rzn"):
            sh = np.zeros((128, 128), np.float32)
            sh[0:H, 0:H] = _gate(pl["Whh"], gi).T
            sh[H:, H:] = _gate(pr["Whh"], gi).T
            c[f"sh{layer}_{gn}"] = sh
            if layer == 1:
                sx = np.zeros((3, 128), np.float32)
                sx[0, 0:H] = _gate(pl["Wih"], gi)[:, 0]
                sx[1, H:] = _gate(pr["Wih"], gi)[:, 0]
                if gn == "n":
                    sx[2, 0:H] = _gate(pl["bih"], gi)
                    sx[2, H:] = _gate(pr["bih"], gi)
                else:
                    sx[2, 0:H] = _gate(pl["bih"], gi) + _gate(pl["bhh"], gi)
                    sx[2, H:] = _gate(pr["bih"], gi) + _gate(pr["bhh"], gi)
                c[f"sx1_{gn}"] = sx
            else:
                sx = np.zeros((128, 128), np.float32)
                sx[0:H, 0:H] = _gate(pl["Wih"], gi).T
                sx[H:, H:] = _gate(pr["Wih"], gi).T
                c[f"sx2_{gn}"] = sx
    l2l, l2r = gl[1], gr[1]
    c["b_r2"] = np.concatenate([_gate(l2l["bih"], 0) + _gate(l2l["bhh"], 0),
                                _gate(l2r["bih"], 0) + _gate(l2r["bhh"], 0)])
    c["b_z2"] = np.concatenate([_gate(l2l["bih"], 1) + _gate(l2l["bhh"], 1),
                                _gate(l2r["bih"], 1) + _gate(l2r["bhh"], 1)])
    c["b_hn1"] = np.concatenate([_gate(gl[0]["bhh"], 2), _gate(gr[0]["bhh"], 2)])
    c["b_hn2"] = np.concatenate([_gate(l2l["bhh"], 2), _gate(l2r["bhh"], 2)])
    c["b_in2"] = np.concatenate([_gate(l2l["bih"], 2), _gate(l2r["bih"], 2)])
    for k in ("b_r2", "b_z2", "b_hn1", "b_hn2", "b_in2"):
        c[k] = c[k].reshape(128, 1)
    c["identb"] = np.eye(128, dtype=np.float32)
    c["sv"] = p["v_w"].T.copy()                       # [128, 64]
    c["smd"] = np.stack([p["mean_w"][0], p["d_w"][0]], axis=1)  # [128, 2]
    c["vb"] = p["v_b"].reshape(R, 1)
    c["id65"] = np.eye(W65, dtype=np.float32)
    c["iaug"] = np.eye(R, W65, dtype=np.float32)
    c["db"] = p["d_b"].reshape(1, 1)
    bf = {"identb", "sv", "smd"} | {f"sh{l}_{g}" for l in (1, 2) for g in "rzn"} \
        | {f"sx{l}_{g}" for l in (1, 2) for g in "rzn"}
    out = {}
    for k, v in c.items():
        out[k] = v.astype(ml_dtypes.bfloat16) if k in bf else v.astype(np.float32)
    return out, float(p["mean_b"][0])


def kernel(x_right, x_left, y, params):
    global last_exec_time_ns
    x_right = np.asarray(x_right, np.float32)
    x_left = np.asarray(x_left, np.float32)
    y = np.asarray(y, np.float32)
    if "nc" not in _CACHE:
        _CACHE["nc"] = _build()
    nc = _CACHE["nc"]
    consts, mean_b = _consts(params)

    in_maps = []
    for core in range(NCORES):
        bs = slice(core * NB, (core + 1) * NB)
        xl = x_left[bs].reshape(N, L)
        xr = x_right[bs].reshape(N, L)
        x3 = np.empty((L, 3, N), np.float32)
        x3[:, 0, :] = xl.T
        x3[:, 1, :] = xr.T
        x3[:, 2, :] = 1.0
        m = dict(consts)
        m["x3"] = x3.astype(ml_dtypes.bfloat16)
        m["yp"] = (y[bs].reshape(1, N) - mean_b).astype(np.float32)
        in_maps.append(m)

    trace = bool(int(os.environ.get("BASS_KERNEL_TRACE", "0")))
    if trace:
        # the axon NTFF hook needs an initialized PJRT client: warm up first
        run_bass_kernel_spmd(nc, in_maps, core_ids=list(range(NCORES)),
                             trace=False)
    res = run_bass_kernel_spmd(nc, in_maps, core_ids=list(range(NCORES)),
                               trace=trace)
    last_exec_time_ns = res.exec_time_ns
    total = sum(float(res.results[c]["out"][0, 0]) for c in range(NCORES))
    return np.float32(total / B)
